# revision 10
# baseline (speedup 1.0000x reference)
"""Multi-head attention (B=2, S=2048, D=1024, H=16) on 8 NeuronCores.

Sharding: tensor-parallel over heads — 2 heads per core. Each core computes
q/k/v projections for its 128 output columns, full attention for its 2 heads
(both batches), and a partial out-projection [4096, 1024]. Host sums the 8
partials and adds the output bias.

Device-side layout choices:
  - Q and K are produced *transposed* ([head_cols, tokens]) straight out of
    the projection matmuls — the operand layout the scores^T matmul wants
    (contract dim = head dim = 64).
  - scores are computed transposed ([keys, q]) so exp applies elementwise and
    attn@V contracts keys on the partition dim — the big P matrix is never
    transposed.
  - V carries an extra all-ones column so attn@[V|1] yields the softmax
    denominator (row 64) along with the unnormalized output (rows 0..63).
  - softmax skips max-subtraction: scores are ~N(0, 0.33^2) by construction.
  - exp runs on 1024-wide tiles (amortizes ACT's ~352-cycle fixed cost); ACT
    does nothing but exp.
  - exp'd scores live in per-(b,h,q-halfgroup) tiles, double-buffered, so the
    scores->exp->attn@V pipeline flows across heads/batches.
  - softmax denominators: DVE reciprocal -> GpSimd partition_broadcast (idle
    engine) -> DVE multiply during PSUM evacuation. PE never blocks on it.
"""

import os
import numpy as np
import ml_dtypes

B, S, D, H = 2, 2048, 1024, 16
HD = D // H          # 64
BS = B * S           # 4096 tokens
NCORES = 8
HPC = H // NCORES    # heads per core = 2
CPC = HPC * HD       # output cols per core = 128
KC = D // 128        # contract chunks = 8
QCH = 512            # matmul moving free dim
NKT = S // 128       # 16 key tiles per batch
QG = 1024            # q-group width (exp tile / et tile width)
NQG = S // QG        # 2 q-groups per batch

BF16 = ml_dtypes.bfloat16

_prog = None


def _build_program():
    import concourse.bacc as bacc
    import concourse.tile as tile
    from concourse import mybir

    f32 = mybir.dt.float32
    bf16 = mybir.dt.bfloat16
    AF = mybir.ActivationFunctionType

    nc = bacc.Bacc("TRN2", debug=False, enable_asserts=False, num_devices=NCORES)

    xT = nc.dram_tensor("xT", [D, BS], bf16, kind="ExternalInput").ap()
    wq = nc.dram_tensor("wq", [D, CPC], bf16, kind="ExternalInput").ap()
    wk = nc.dram_tensor("wk", [D, CPC], bf16, kind="ExternalInput").ap()
    wv = nc.dram_tensor("wv", [D, CPC], bf16, kind="ExternalInput").ap()
    wo = nc.dram_tensor("wo", [CPC, D], bf16, kind="ExternalInput").ap()
    bq = nc.dram_tensor("bq", [CPC, 1], f32, kind="ExternalInput").ap()
    bk = nc.dram_tensor("bk", [CPC, 1], f32, kind="ExternalInput").ap()
    bv = nc.dram_tensor("bv", [1, CPC], bf16, kind="ExternalInput").ap()
    out = nc.dram_tensor("out", [BS, D], f32, kind="ExternalOutput").ap()

    SCALE = float(1.0 / np.sqrt(HD))

    with tile.TileContext(nc) as tc:
        with (
            tc.tile_pool(name="big", bufs=1) as big,
            tc.tile_pool(name="sm", bufs=1) as sm,
            tc.tile_pool(name="attn", bufs=2) as attn,
            tc.tile_pool(name="etp", bufs=2) as etp,
            tc.tile_pool(name="ostage", bufs=4) as ostage,
            tc.tile_pool(name="ps", bufs=2, space="PSUM") as ps,
        ):
            # ---- resident SBUF tensors ----
            xt_sb = big.tile([128, KC, BS], bf16, name="xt_sb", tag="xt")
            qt_sb = big.tile([128, BS], bf16, name="qt_sb", tag="qt")
            kt_sb = big.tile([128, BS], bf16, name="kt_sb", tag="kt")
            # V|ones per head: [keys(128) x keytile(32) x (64 V + 1 ones)*2]
            v_sb = big.tile([128, B * NKT, 2 * (HD + 1)], bf16, name="v_sb", tag="v")
            wo_sb = big.tile([128, D], bf16, name="wo_sb", tag="wo")

            wq_sb = sm.tile([128, KC, CPC], bf16, name="wq_sb", tag="wq")
            wk_sb = sm.tile([128, KC, CPC], bf16, name="wk_sb", tag="wk")
            wv_sb = sm.tile([128, KC, CPC], bf16, name="wv_sb", tag="wv")
            bq_sb = sm.tile([CPC, 1], f32, name="bq_sb", tag="bq")
            bk_sb = sm.tile([CPC, 1], f32, name="bk_sb", tag="bk")
            bv_sb = sm.tile([1, CPC], bf16, name="bv_sb", tag="bv")
            ones_bf = sm.tile([1, 128], bf16, name="ones_bf", tag="onesb")

            nc.vector.memset(ones_bf, 1.0)
            nc.vector.memset(v_sb[:, :, HD : HD + 1], 1.0)
            nc.vector.memset(v_sb[:, :, 2 * HD + 1 : 2 * HD + 2], 1.0)

            # first QK-proj tile needs only wq/wk + token-block 0 of xT:
            # emit those DMAs first so PE starts ASAP
            xt_r = xT.rearrange("(c p) n -> p c n", p=128)
            nc.sync.dma_start(out=wq_sb, in_=wq.rearrange("(c p) n -> p c n", p=128))
            nc.sync.dma_start(out=wk_sb, in_=wk.rearrange("(c p) n -> p c n", p=128))
            for c in range(KC):
                nc.sync.dma_start(out=xt_sb[:, c, 0:1024], in_=xt_r[:, c, 0:1024])
            nc.sync.dma_start(out=wv_sb, in_=wv.rearrange("(c p) n -> p c n", p=128))
            nc.sync.dma_start(out=wo_sb, in_=wo)
            nc.sync.dma_start(out=bq_sb, in_=bq)
            nc.sync.dma_start(out=bk_sb, in_=bk)
            nc.sync.dma_start(out=bv_sb, in_=bv)
            for tb in range(1, BS // 1024):
                for c in range(KC):
                    nc.sync.dma_start(
                        out=xt_sb[:, c, tb * 1024 : (tb + 1) * 1024],
                        in_=xt_r[:, c, tb * 1024 : (tb + 1) * 1024],
                    )

            # ---- emission helpers (PE stream is in-order: interleave
            # independent work into semaphore-gated stretches) ----
            from collections import deque

            def qkproj_units(name, w_sb, b_sb, dst, t):
                box = {}

                def emit_half(chalf):
                    def emit():
                        if chalf == 0:
                            box["pp"] = ps.tile(
                                [128, 1024], f32, name=f"pp_{name}{t}", tag="sp"
                            )
                        pp = box["pp"]
                        for c in range(chalf * 4, chalf * 4 + 4):
                            for half in range(2):
                                nc.tensor.matmul(
                                    pp[:, half * QCH : (half + 1) * QCH],
                                    lhsT=w_sb[:, c, :],
                                    rhs=xt_sb[:, c, t * 1024 + half * QCH : t * 1024 + (half + 1) * QCH],
                                    start=(c == 0),
                                    stop=(c == KC - 1),
                                )
                        if chalf == 1:
                            nc.vector.tensor_scalar_add(
                                dst[:, t * 1024 : (t + 1) * 1024], pp, b_sb
                            )
                    return emit

                return [emit_half(0), emit_half(1)]

            def vproj_unit(kt):
                def emit():
                    vp = ps.tile([128, CPC], f32, name=f"vp{kt}", tag="small", bufs=4)
                    for c in range(KC):
                        nc.tensor.matmul(
                            vp,
                            lhsT=xt_sb[:, c, kt * 128 : (kt + 1) * 128],
                            rhs=wv_sb[:, c, :],
                            start=(c == 0),
                            stop=False,
                        )
                    nc.tensor.matmul(vp, lhsT=ones_bf, rhs=bv_sb, start=False, stop=True)
                    nc.vector.tensor_copy(
                        v_sb[:, kt, :].rearrange("p (h c) -> p h c", h=2)[:, :, 0:HD],
                        vp.rearrange("p (h c) -> p h c", h=2),
                    )
                return emit

            def o_units(b, h, qg, qc, et, ot_sb):
                box = {}
                hp = h * HD

                def emit_mm(k0):
                    def emit():
                        if k0 == 0:
                            box["op"] = ps.tile(
                                [HD + 1, QCH], f32,
                                name=f"op{b}{h}{qg}{qc}", tag="small", bufs=4,
                            )
                        op = box["op"]
                        for kt in range(k0, k0 + 4):
                            nc.tensor.matmul(
                                op,
                                lhsT=v_sb[:, b * NKT + kt, h * (HD + 1) : (h + 1) * (HD + 1)],
                                rhs=et[:, kt, qc * QCH : (qc + 1) * QCH],
                                start=(kt == 0),
                                stop=(kt == NKT - 1),
                            )
                    return emit

                def emit_chain():
                    op = box["op"]
                    rc = ostage.tile([1, QCH], f32, name=f"rc{b}{h}{qg}{qc}", tag="rc")
                    nc.vector.reciprocal(rc, op[HD : HD + 1, :])
                    rbs = ostage.tile([HD, QCH], f32, name=f"rbs{b}{h}{qg}{qc}", tag="rbs")
                    nc.gpsimd.partition_broadcast(rbs, rc)
                    nc.vector.tensor_mul(
                        ot_sb[hp : hp + HD, qg * QG + qc * QCH : qg * QG + (qc + 1) * QCH],
                        op[0:HD, :],
                        rbs,
                    )

                return [emit_mm(k0) for k0 in range(0, NKT, 4)] + [emit_chain]

            def outproj_unit(b, qt, ot_sb):
                def emit():
                    for nh in range(2):
                        pq = ps.tile([128, QCH], f32, name=f"pq{b}{qt}{nh}", tag="small", bufs=4)
                        nc.tensor.matmul(
                            pq,
                            lhsT=ot_sb[:, qt * 128 : (qt + 1) * 128],
                            rhs=wo_sb[:, nh * QCH : (nh + 1) * QCH],
                            start=True,
                            stop=True,
                        )
                        os_ = ostage.tile([128, QCH], f32, name=f"os{b}{qt}{nh}", tag="os")
                        nc.vector.tensor_copy(os_, pq)
                        nc.sync.dma_start(
                            out=out[
                                b * S + qt * 128 : b * S + (qt + 1) * 128,
                                nh * QCH : (nh + 1) * QCH,
                            ],
                            in_=os_,
                        )
                return emit

            fill = deque()

            # batch-0 projections emitted inline; batch-1's become fillers
            for t in range(2):
                for u in qkproj_units("q", wq_sb, bq_sb, qt_sb, t):
                    u()
                for u in qkproj_units("k", wk_sb, bk_sb, kt_sb, t):
                    u()
            for kt in range(NKT):
                vproj_unit(kt)()
            for t in range(2, 4):
                fill.extend(qkproj_units("q", wq_sb, bq_sb, qt_sb, t))
                fill.extend(qkproj_units("k", wk_sb, bk_sb, kt_sb, t))
            for kt in range(NKT, 2 * NKT):
                fill.append(vproj_unit(kt))

            # ---- attention ----
            for b in range(B):
                ot_sb = attn.tile([128, S], bf16, name=f"ot{b}", tag="ot")
                for h in range(HPC):
                    hp = h * HD
                    for qg in range(NQG):
                        q0 = b * S + qg * QG
                        et = etp.tile([128, NKT, QG], bf16, name=f"et{b}{h}{qg}", tag="et")
                        for kt in range(NKT):
                            sp = ps.tile([128, QG], f32, name=f"sp{b}{h}{qg}{kt}", tag="sp")
                            for qh in range(2):
                                nc.tensor.matmul(
                                    sp[:, qh * QCH : (qh + 1) * QCH],
                                    lhsT=kt_sb[hp : hp + HD, b * S + kt * 128 : b * S + (kt + 1) * 128],
                                    rhs=qt_sb[hp : hp + HD, q0 + qh * QCH : q0 + (qh + 1) * QCH],
                                    start=True,
                                    stop=True,
                                )
                            nc.scalar.activation(et[:, kt, :], sp, AF.Exp, scale=SCALE)
                            if fill:
                                fill.popleft()()
                        for qc in range(QG // QCH):
                            fill.extend(o_units(b, h, qg, qc, et, ot_sb))
                        if h == HPC - 1:
                            # out-projection for the q-tiles this qg covers
                            # becomes available once both heads' muls land
                            for qt in range(qg * (QG // 128), (qg + 1) * (QG // 128)):
                                fill.append(outproj_unit(b, qt, ot_sb))
            while fill:
                fill.popleft()()
    nc.compile()
    return nc


def _get_prog():
    global _prog
    if _prog is None:
        _prog = _build_program()
    return _prog


def kernel(x, Wq, bq, Wk, bk, Wv, bv, Wo, bo):
    from concourse import bass_utils

    nc = _get_prog()

    xT = np.ascontiguousarray(
        np.asarray(x, dtype=np.float32).reshape(BS, D).T
    ).astype(BF16)

    in_maps = []
    for c in range(NCORES):
        cols = slice(c * CPC, (c + 1) * CPC)
        in_maps.append(
            {
                "xT": xT,
                "wq": np.ascontiguousarray(Wq[cols, :].T).astype(BF16),
                "wk": np.ascontiguousarray(Wk[cols, :].T).astype(BF16),
                "wv": np.ascontiguousarray(Wv[cols, :].T).astype(BF16),
                "wo": np.ascontiguousarray(Wo[:, cols].T).astype(BF16),
                "bq": np.asarray(bq[cols], np.float32).reshape(CPC, 1),
                "bk": np.asarray(bk[cols], np.float32).reshape(CPC, 1),
                "bv": np.asarray(bv[cols], np.float32).reshape(1, CPC).astype(BF16),
            }
        )

    res = bass_utils.run_bass_kernel_spmd(
        nc,
        in_maps,
        core_ids=list(range(NCORES)),
        trace=bool(int(os.environ.get("KERNEL_TRACE", "0"))),
    )
    kernel.last_results = res

    acc = np.zeros((BS, D), np.float64)
    for c in range(NCORES):
        acc += res.results[c]["out"].astype(np.float64)
    acc += np.asarray(bo, np.float64)[None, :]
    return acc.reshape(B, S, D).astype(np.float32)


# revision 11
# speedup vs baseline: 1.0020x; 1.0020x over previous
"""Multi-head attention (B=2, S=2048, D=1024, H=16) on 8 NeuronCores.

Sharding: tensor-parallel over heads — 2 heads per core. Each core computes
q/k/v projections for its 128 output columns, full attention for its 2 heads
(both batches), and a partial out-projection [4096, 1024]. Host sums the 8
partials and adds the output bias.

Device-side layout choices:
  - Q and K are produced *transposed* ([head_cols, tokens]) straight out of
    the projection matmuls — the operand layout the scores^T matmul wants
    (contract dim = head dim = 64).
  - scores are computed transposed ([keys, q]) so exp applies elementwise and
    attn@V contracts keys on the partition dim — the big P matrix is never
    transposed.
  - V carries an extra all-ones column so attn@[V|1] yields the softmax
    denominator (row 64) along with the unnormalized output (rows 0..63).
  - softmax skips max-subtraction: scores are ~N(0, 0.33^2) by construction.
  - exp runs on 1024-wide tiles (amortizes ACT's ~352-cycle fixed cost); ACT
    does nothing but exp.
  - exp'd scores live in per-(b,h,q-halfgroup) tiles, double-buffered, so the
    scores->exp->attn@V pipeline flows across heads/batches.
  - softmax denominators: DVE reciprocal -> GpSimd partition_broadcast (idle
    engine) -> DVE multiply during PSUM evacuation. PE never blocks on it.
"""

import os
import numpy as np
import ml_dtypes

B, S, D, H = 2, 2048, 1024, 16
HD = D // H          # 64
BS = B * S           # 4096 tokens
NCORES = 8
HPC = H // NCORES    # heads per core = 2
CPC = HPC * HD       # output cols per core = 128
KC = D // 128        # contract chunks = 8
QCH = 512            # matmul moving free dim
NKT = S // 128       # 16 key tiles per batch
QG = 1024            # q-group width (exp tile / et tile width)
NQG = S // QG        # 2 q-groups per batch

BF16 = ml_dtypes.bfloat16

_prog = None


def _build_program():
    import concourse.bacc as bacc
    import concourse.tile as tile
    from concourse import mybir

    f32 = mybir.dt.float32
    bf16 = mybir.dt.bfloat16
    AF = mybir.ActivationFunctionType

    nc = bacc.Bacc("TRN2", debug=False, enable_asserts=False, num_devices=NCORES)

    xT = nc.dram_tensor("xT", [D, BS], bf16, kind="ExternalInput").ap()
    wq = nc.dram_tensor("wq", [D, CPC], bf16, kind="ExternalInput").ap()
    wk = nc.dram_tensor("wk", [D, CPC], bf16, kind="ExternalInput").ap()
    wv = nc.dram_tensor("wv", [D, CPC], bf16, kind="ExternalInput").ap()
    wo = nc.dram_tensor("wo", [CPC, D], bf16, kind="ExternalInput").ap()
    bq = nc.dram_tensor("bq", [CPC, 1], f32, kind="ExternalInput").ap()
    bk = nc.dram_tensor("bk", [CPC, 1], f32, kind="ExternalInput").ap()
    bv = nc.dram_tensor("bv", [1, CPC], bf16, kind="ExternalInput").ap()
    out = nc.dram_tensor("out", [BS, D], f32, kind="ExternalOutput").ap()

    SCALE = float(1.0 / np.sqrt(HD))

    with tile.TileContext(nc) as tc:
        with (
            tc.tile_pool(name="big", bufs=1) as big,
            tc.tile_pool(name="sm", bufs=1) as sm,
            tc.tile_pool(name="attn", bufs=2) as attn,
            tc.tile_pool(name="etp", bufs=2) as etp,
            tc.tile_pool(name="ostage", bufs=4) as ostage,
            tc.tile_pool(name="ps", bufs=2, space="PSUM") as ps,
        ):
            # ---- resident SBUF tensors ----
            xt_sb = big.tile([128, KC, BS], bf16, name="xt_sb", tag="xt")
            qt_sb = big.tile([128, BS], bf16, name="qt_sb", tag="qt")
            kt_sb = big.tile([128, BS], bf16, name="kt_sb", tag="kt")
            # V|ones per head: [keys(128) x keytile(32) x (64 V + 1 ones)*2]
            v_sb = big.tile([128, B * NKT, 2 * (HD + 1)], bf16, name="v_sb", tag="v")
            wo_sb = big.tile([128, D], bf16, name="wo_sb", tag="wo")

            wq_sb = sm.tile([128, KC, CPC], bf16, name="wq_sb", tag="wq")
            wk_sb = sm.tile([128, KC, CPC], bf16, name="wk_sb", tag="wk")
            wv_sb = sm.tile([128, KC, CPC], bf16, name="wv_sb", tag="wv")
            bq_sb = sm.tile([CPC, 1], f32, name="bq_sb", tag="bq")
            bk_sb = sm.tile([CPC, 1], f32, name="bk_sb", tag="bk")
            bv_sb = sm.tile([1, CPC], bf16, name="bv_sb", tag="bv")
            ones_bf = sm.tile([1, 128], bf16, name="ones_bf", tag="onesb")

            nc.vector.memset(ones_bf, 1.0)
            nc.vector.memset(v_sb[:, :, HD : HD + 1], 1.0)
            nc.vector.memset(v_sb[:, :, 2 * HD + 1 : 2 * HD + 2], 1.0)

            # first QK-proj tile needs only wq/wk + token-block 0 of xT:
            # emit those DMAs first so PE starts ASAP
            xt_r = xT.rearrange("(c p) n -> p c n", p=128)
            nc.sync.dma_start(out=wq_sb, in_=wq.rearrange("(c p) n -> p c n", p=128))
            nc.sync.dma_start(out=wk_sb, in_=wk.rearrange("(c p) n -> p c n", p=128))
            for c in range(KC):
                nc.sync.dma_start(out=xt_sb[:, c, 0:1024], in_=xt_r[:, c, 0:1024])
            nc.sync.dma_start(out=wv_sb, in_=wv.rearrange("(c p) n -> p c n", p=128))
            nc.sync.dma_start(out=wo_sb, in_=wo)
            nc.sync.dma_start(out=bq_sb, in_=bq)
            nc.sync.dma_start(out=bk_sb, in_=bk)
            nc.sync.dma_start(out=bv_sb, in_=bv)
            for tb in range(1, BS // 1024):
                for c in range(KC):
                    nc.sync.dma_start(
                        out=xt_sb[:, c, tb * 1024 : (tb + 1) * 1024],
                        in_=xt_r[:, c, tb * 1024 : (tb + 1) * 1024],
                    )

            # ---- emission helpers (PE stream is in-order: interleave
            # independent work into semaphore-gated stretches) ----
            from collections import deque

            def qkproj_units(name, w_sb, b_sb, dst, t):
                box = {}

                def emit_half(chalf):
                    def emit():
                        if chalf == 0:
                            box["pp"] = ps.tile(
                                [128, 1024], f32, name=f"pp_{name}{t}", tag="sp"
                            )
                        pp = box["pp"]
                        for c in range(chalf * 4, chalf * 4 + 4):
                            for half in range(2):
                                nc.tensor.matmul(
                                    pp[:, half * QCH : (half + 1) * QCH],
                                    lhsT=w_sb[:, c, :],
                                    rhs=xt_sb[:, c, t * 1024 + half * QCH : t * 1024 + (half + 1) * QCH],
                                    start=(c == 0),
                                    stop=(c == KC - 1),
                                )
                        if chalf == 1:
                            nc.vector.tensor_scalar_add(
                                dst[:, t * 1024 : (t + 1) * 1024], pp, b_sb
                            )
                    return emit

                return [emit_half(0), emit_half(1)]

            def vproj_unit(kt):
                def emit():
                    vp = ps.tile([128, CPC], f32, name=f"vp{kt}", tag="small", bufs=4)
                    for c in range(KC):
                        nc.tensor.matmul(
                            vp,
                            lhsT=xt_sb[:, c, kt * 128 : (kt + 1) * 128],
                            rhs=wv_sb[:, c, :],
                            start=(c == 0),
                            stop=False,
                        )
                    nc.tensor.matmul(vp, lhsT=ones_bf, rhs=bv_sb, start=False, stop=True)
                    nc.vector.tensor_copy(
                        v_sb[:, kt, :].rearrange("p (h c) -> p h c", h=2)[:, :, 0:HD],
                        vp.rearrange("p (h c) -> p h c", h=2),
                    )
                return emit

            def o_units(b, h, qg, qc, et, ot_sb):
                box = {}
                hp = h * HD

                def emit_mm(k0):
                    def emit():
                        if k0 == 0:
                            box["op"] = ps.tile(
                                [HD + 1, QCH], f32,
                                name=f"op{b}{h}{qg}{qc}", tag="small", bufs=4,
                            )
                        op = box["op"]
                        for kt in range(k0, k0 + 4):
                            nc.tensor.matmul(
                                op,
                                lhsT=v_sb[:, b * NKT + kt, h * (HD + 1) : (h + 1) * (HD + 1)],
                                rhs=et[:, kt, qc * QCH : (qc + 1) * QCH],
                                start=(kt == 0),
                                stop=(kt == NKT - 1),
                            )
                    return emit

                def emit_chain():
                    op = box["op"]
                    rc = ostage.tile([1, QCH], f32, name=f"rc{b}{h}{qg}{qc}", tag="rc")
                    nc.vector.reciprocal(rc, op[HD : HD + 1, :])
                    rbs = ostage.tile([HD, QCH], f32, name=f"rbs{b}{h}{qg}{qc}", tag="rbs")
                    nc.gpsimd.partition_broadcast(rbs, rc)
                    nc.vector.tensor_mul(
                        ot_sb[hp : hp + HD, qg * QG + qc * QCH : qg * QG + (qc + 1) * QCH],
                        op[0:HD, :],
                        rbs,
                    )

                return [emit_mm(k0) for k0 in range(0, NKT, 4)] + [emit_chain]

            def outproj_unit(b, qt, ot_sb):
                def emit():
                    os_ = ostage.tile([128, 1024], f32, name=f"os{b}{qt}", tag="os", bufs=3)
                    for nh in range(2):
                        pq = ps.tile([128, QCH], f32, name=f"pq{b}{qt}{nh}", tag="small", bufs=4)
                        nc.tensor.matmul(
                            pq,
                            lhsT=ot_sb[:, qt * 128 : (qt + 1) * 128],
                            rhs=wo_sb[:, nh * QCH : (nh + 1) * QCH],
                            start=True,
                            stop=True,
                        )
                        nc.vector.tensor_copy(os_[:, nh * QCH : (nh + 1) * QCH], pq)
                    nc.sync.dma_start(
                        out=out[b * S + qt * 128 : b * S + (qt + 1) * 128, :],
                        in_=os_,
                    )
                return emit

            fill = deque()

            # batch-0 projections emitted inline; batch-1's become fillers
            for t in range(2):
                for u in qkproj_units("q", wq_sb, bq_sb, qt_sb, t):
                    u()
                for u in qkproj_units("k", wk_sb, bk_sb, kt_sb, t):
                    u()
            for kt in range(NKT):
                vproj_unit(kt)()
            for t in range(2, 4):
                fill.extend(qkproj_units("q", wq_sb, bq_sb, qt_sb, t))
                fill.extend(qkproj_units("k", wk_sb, bk_sb, kt_sb, t))
            for kt in range(NKT, 2 * NKT):
                fill.append(vproj_unit(kt))

            # ---- attention ----
            for b in range(B):
                ot_sb = attn.tile([128, S], bf16, name=f"ot{b}", tag="ot")
                for h in range(HPC):
                    hp = h * HD
                    for qg in range(NQG):
                        q0 = b * S + qg * QG
                        et = etp.tile([128, NKT, QG], bf16, name=f"et{b}{h}{qg}", tag="et")
                        for kt in range(NKT):
                            sp = ps.tile([128, QG], f32, name=f"sp{b}{h}{qg}{kt}", tag="sp")
                            for qh in range(2):
                                nc.tensor.matmul(
                                    sp[:, qh * QCH : (qh + 1) * QCH],
                                    lhsT=kt_sb[hp : hp + HD, b * S + kt * 128 : b * S + (kt + 1) * 128],
                                    rhs=qt_sb[hp : hp + HD, q0 + qh * QCH : q0 + (qh + 1) * QCH],
                                    start=True,
                                    stop=True,
                                )
                            nc.scalar.activation(et[:, kt, :], sp, AF.Exp, scale=SCALE)
                            if fill:
                                fill.popleft()()
                            if len(fill) > 8:
                                fill.popleft()()
                        for qc in range(QG // QCH):
                            fill.extend(o_units(b, h, qg, qc, et, ot_sb))
                        if h == HPC - 1:
                            # out-projection for the q-tiles this qg covers
                            # becomes available once both heads' muls land
                            for qt in range(qg * (QG // 128), (qg + 1) * (QG // 128)):
                                fill.append(outproj_unit(b, qt, ot_sb))
            while fill:
                fill.popleft()()
    nc.compile()
    return nc


def _get_prog():
    global _prog
    if _prog is None:
        _prog = _build_program()
    return _prog


def kernel(x, Wq, bq, Wk, bk, Wv, bv, Wo, bo):
    from concourse import bass_utils

    nc = _get_prog()

    xT = np.ascontiguousarray(
        np.asarray(x, dtype=np.float32).reshape(BS, D).T
    ).astype(BF16)

    in_maps = []
    for c in range(NCORES):
        cols = slice(c * CPC, (c + 1) * CPC)
        in_maps.append(
            {
                "xT": xT,
                "wq": np.ascontiguousarray(Wq[cols, :].T).astype(BF16),
                "wk": np.ascontiguousarray(Wk[cols, :].T).astype(BF16),
                "wv": np.ascontiguousarray(Wv[cols, :].T).astype(BF16),
                "wo": np.ascontiguousarray(Wo[:, cols].T).astype(BF16),
                "bq": np.asarray(bq[cols], np.float32).reshape(CPC, 1),
                "bk": np.asarray(bk[cols], np.float32).reshape(CPC, 1),
                "bv": np.asarray(bv[cols], np.float32).reshape(1, CPC).astype(BF16),
            }
        )

    res = bass_utils.run_bass_kernel_spmd(
        nc,
        in_maps,
        core_ids=list(range(NCORES)),
        trace=bool(int(os.environ.get("KERNEL_TRACE", "0"))),
    )
    kernel.last_results = res

    acc = np.zeros((BS, D), np.float64)
    for c in range(NCORES):
        acc += res.results[c]["out"].astype(np.float64)
    acc += np.asarray(bo, np.float64)[None, :]
    return acc.reshape(B, S, D).astype(np.float32)
